# revision 1
# baseline (speedup 1.0000x reference)
"""BitLinear (ternary-weight linear) Trainium2 kernel — fp8 DoubleRow version.

Math (matching the reference):
    s      = max(act_scale, 1e-5)
    z      = clip(round(x / s), -127, 127)           # int8-valued
    out    = (alpha * s) * (z @ sign(W).T) + bias

TRN2's fp8 DoubleRow matmul contracts 2 k-tiles (256 deep) per
instruction at 0.5 cycles per output row -> 4x bf16 MAC throughput.
z in [-127,127] is not exact in fp8e4 (4 significand bits), so k-tiles
are handled two ways:
  - EX_K exact tiles:  h   = round(z/16)  (any rounding mode works)
                       h16 = 16*h  in {-128..128 step 16} -> exact fp8e4
                       l   = z - h16  in [-15,15]         -> exact fp8e4
                       two DoubleRow passes, exact integer math in the
                       f32 PSUM accumulator.
  - SINGLE_K tiles:    s8 = fp8e4(z) single pass (~2^-4 relative rounding
                       on |z|>16). With a 16/16 split the end-to-end rel
                       err is 1.46e-2 (measured on the real inputs, incl.
                       bf16 output store) against the 2e-2 gate.
PE work: (16*2 + 16) DoubleRow instr per psum group = 328us/core.

Engine/ring plan (in-order sequencers make ring assignment = scheduling;
fill engines must not share a queue with matmul-phase consumers, or the
next rep's fill serializes behind this rep's drain):
    ACT   : quant round-scale, h-round, s8 convert   (fill only, no DMA)
    DVE   : clamp, h16 = 16h -> fp8, l = z - h16     (fill only, no DMA)
    Pool  : psum*alpha_s + bias -> bf16 drain; SWDGE ring for z stores
            and osb stores                           (matmul phase)
    SP    : x loads, xbar transposes, wt loads       (HWDGE ring)
    PE    : 3072 DoubleRow matmuls
The hl pool holds 5 quarter-slots per tag so the next rep's conversion
can start while this rep's matmuls still read older quarters.
"""

import sys

sys.path.insert(0, "/opt/trn_rl_repo")

import numpy as np
import ml_dtypes

# ---- problem constants (hardcoded per harness contract) ----
B, S, IN, OUT = 4, 4096, 4096, 4096
TOKENS = B * S              # 16384
N_CORES = 8
T = TOKENS // N_CORES       # 2048 tokens per core
KT = IN // 128              # 32 k-tiles (contraction)
EX_K = 16                   # exact k-tiles (h16 + l passes)
SINGLE_K = KT - EX_K        # single-pass k-tiles (plain fp8 quant)
N_CHUNK = 512               # output columns per PSUM tile
NT = OUT // N_CHUNK         # 8 n-chunks
Q = 512                     # token-quarter (transpose/convert granularity)
NQ = T // Q                 # 4 quarters
MT = Q // 128               # 4 m-tiles per quarter
XCH = 1024                  # free-dim chunk for quantization staging
PIPE_Q = 1                  # quarters of the next rep converted during B


def _build_program(inv_s: float, alpha_s: float, reps: int = 1,
                   bonly: bool = False):
    import concourse.mybir as mybir
    import concourse.tile as tile
    from concourse import bacc

    nc = bacc.Bacc("TRN2", target_bir_lowering=False, debug=False,
                   num_devices=N_CORES)

    x_d = nc.dram_tensor("x", [T, IN], mybir.dt.float32, kind="ExternalInput")
    # wt[p, n, k, c] = sign(W)[n*512 + c, k*128 + p]
    wt_d = nc.dram_tensor("wt", [128, NT, KT, N_CHUNK], mybir.dt.float8e4,
                          kind="ExternalInput")
    bias_d = nc.dram_tensor("bias", [128, OUT], mybir.dt.bfloat16,
                            kind="ExternalInput")
    out_d = nc.dram_tensor("out", [T, OUT], mybir.dt.bfloat16,
                           kind="ExternalOutput")
    z_d = nc.dram_tensor("z_scratch", [T, IN], mybir.dt.int16)

    AF = mybir.ActivationFunctionType
    ALU = mybir.AluOpType
    DR = mybir.MatmulPerfMode.DoubleRow

    with tile.TileContext(nc) as tc:
        with (
            tc.tile_pool(name="xstage", bufs=2) as xstage,
            tc.tile_pool(name="zstage", bufs=2) as zstage,
            tc.tile_pool(name="ztp", bufs=16) as ztp,
            tc.tile_pool(name="hp", bufs=4) as hp,
            tc.tile_pool(name="hl", bufs=NQ + PIPE_Q) as hl_pool,
            tc.tile_pool(name="wtp", bufs=2) as wt_pool,
            tc.tile_pool(name="outsb", bufs=6) as out_pool,
            tc.tile_pool(name="biasp", bufs=1) as bias_pool,
            tc.tile_pool(name="psum", bufs=8, space="PSUM") as psum_pool,
        ):
            bias_t = bias_pool.tile([128, OUT], mybir.dt.bfloat16, tag="bias")
            nc.sync.dma_start(bias_t[:], bias_d.ap())

            def emit_quant(m):
                """Quantize one 128-token row block: x -> round/clip ->
                int16, bounce to DRAM."""
                r0 = m * 128
                for c in range(IN // XCH):
                    i0 = c * XCH
                    xt = xstage.tile([128, XCH], mybir.dt.float32, tag="xf32")
                    nc.sync.dma_start(xt[:],
                                      x_d.ap()[r0:r0 + 128, i0:i0 + XCH])
                    z0 = zstage.tile([128, XCH], mybir.dt.int16, tag="z0")
                    nc.scalar.activation(z0[:], xt[:], AF.Copy,
                                         bias=0.0, scale=float(inv_s))
                    nc.sync.dma_start(z_d.ap()[r0:r0 + 128, i0:i0 + XCH],
                                      z0[:])

            def emit_convert(q, tiles=None, ks=None):
                """Transpose quarter q k-major; exact tiles -> h16/l fp8,
                single-pass tiles -> s8 fp8. ks limits the k range so the
                work can be interleaved into the previous rep's passes."""
                t0 = q * Q
                if tiles is None:
                    tiles = (
                        hl_pool.tile([128, EX_K, Q], mybir.dt.float8e4,
                                     tag="h16", name="h16t"),
                        hl_pool.tile([128, EX_K, Q], mybir.dt.float8e4,
                                     tag="l", name="lt"),
                        hl_pool.tile([128, SINGLE_K, Q], mybir.dt.float8e4,
                                     tag="s8", name="s8t"),
                    )
                h16_t, l_t, s8_t = tiles
                for k in (range(KT) if ks is None else ks):
                    zT = ztp.tile([128, Q], mybir.dt.int16, tag="zT")
                    nc.sync.dma_start_transpose(
                        zT[:], z_d.ap()[t0:t0 + Q, k * 128:(k + 1) * 128])
                    if k < EX_K:
                        zc = hp.tile([128, Q], mybir.dt.int16, tag="zc")
                        nc.vector.tensor_scalar(zc[:], zT[:], 127.0, -127.0,
                                                ALU.min, ALU.max)
                        h = hp.tile([128, Q], mybir.dt.int16, tag="h")
                        nc.scalar.activation(h[:], zc[:], AF.Copy,
                                             bias=0.0, scale=0.0625)
                        nc.vector.tensor_scalar(h16_t[:, k, :], h[:], 16.0,
                                                None, ALU.mult)
                        nc.vector.tensor_tensor(l_t[:, k, :], zc[:],
                                                h16_t[:, k, :], ALU.subtract)
                    else:
                        nc.vector.tensor_scalar(s8_t[:, k - EX_K, :], zT[:],
                                                127.0, -127.0,
                                                ALU.min, ALU.max)
                return (h16_t, l_t, s8_t)

            def load_wt(n):
                wt = wt_pool.tile([128, KT, N_CHUNK], mybir.dt.float8e4,
                                  tag="wt")
                nc.sync.dma_start(wt[:], wt_d.ap()[:, n, :, :])
                return wt

            hls = None
            nhls = None
            for _rep in range(reps):
                wts = {0: load_wt(0), 1: load_wt(1)}
                if hls is None:
                    hls = []
                    for q in range(NQ):
                        for mm in range(MT):
                            emit_quant(q * MT + mm)
                        hls.append(emit_convert(q))
                elif not bonly:
                    pass

                for n in range(NT):
                    wt = wts.pop(n)
                    for q in range(NQ):
                        h16_t, l_t, s8_t = hls[q]
                        for mm in range(MT):
                            ms = slice(mm * 128, (mm + 1) * 128)
                            psum = psum_pool.tile([128, N_CHUNK],
                                                  mybir.dt.float32)
                            for kk in range(EX_K // 2):
                                ks = slice(2 * kk, 2 * kk + 2)
                                nc.tensor.matmul(
                                    psum[:], h16_t[:, ks, ms], wt[:, ks, :],
                                    start=(kk == 0), stop=False,
                                    perf_mode=DR)
                            for kk in range(EX_K // 2):
                                ks = slice(2 * kk, 2 * kk + 2)
                                nc.tensor.matmul(
                                    psum[:], l_t[:, ks, ms], wt[:, ks, :],
                                    start=False, stop=False, perf_mode=DR)
                            for kk in range(SINGLE_K // 2):
                                ks = slice(2 * kk, 2 * kk + 2)
                                kw = slice(EX_K + 2 * kk, EX_K + 2 * kk + 2)
                                nc.tensor.matmul(
                                    psum[:], s8_t[:, ks, ms], wt[:, kw, :],
                                    start=False,
                                    stop=(kk == SINGLE_K // 2 - 1),
                                    perf_mode=DR)
                            osb = out_pool.tile([128, N_CHUNK],
                                                mybir.dt.bfloat16, tag="osb")
                            nc.vector.scalar_tensor_tensor(
                                osb[:], psum[:], float(alpha_s),
                                bias_t[:, n * N_CHUNK:(n + 1) * N_CHUNK],
                                ALU.mult, ALU.add)
                            t0 = q * Q + mm * 128
                            nc.gpsimd.dma_start(
                                out_d.ap()[t0:t0 + 128,
                                           n * N_CHUNK:(n + 1) * N_CHUNK],
                                osb[:])
                    if n + 2 < NT:
                        wts[n + 2] = load_wt(n + 2)
                    if not bonly and _rep + 1 < reps:
                        if n < NQ:
                            for mm in range(MT):
                                emit_quant(n * MT + mm)
                        else:
                            if nhls is None:
                                nhls = [None] * NQ
                            b = n - NQ          # 0..3
                            for qq in range(PIPE_Q):
                                nhls[qq] = emit_convert(
                                    qq, tiles=nhls[qq],
                                    ks=range(b * KT // 4,
                                             (b + 1) * KT // 4))
                if not bonly and nhls is not None:
                    for qq in range(PIPE_Q, NQ):
                        nhls[qq] = emit_convert(qq)
                    hls = nhls
                    nhls = None

    nc.compile()
    return nc


def prep_scalars(alpha, act_scale):
    s = max(float(np.asarray(act_scale)), 1e-5)
    inv_s = 1.0 / np.float32(s)
    alpha_s = float(np.float32(np.asarray(alpha, dtype=np.float32)) *
                    np.float32(s))
    return float(inv_s), alpha_s


def prep_weights(packed_w, bias):
    """Host-side weight/bias packing (replicated across cores)."""
    w_sign = np.asarray(packed_w, dtype=np.float32) - 1.0     # [OUT, IN]
    # wt[p, n, k, c] = w_sign[n*512 + c, k*128 + p]
    wt = w_sign.reshape(NT, N_CHUNK, KT, 128).transpose(3, 0, 2, 1)
    whost = np.ascontiguousarray(wt.astype(ml_dtypes.float8_e4m3))
    bias_rep = np.ascontiguousarray(
        np.broadcast_to(
            np.asarray(bias, dtype=np.float32).astype(ml_dtypes.bfloat16)
            [None, :], (128, OUT)))
    return whost, bias_rep


def kernel(x, packed_w, alpha, act_scale, bias, _trace=False):
    from concourse.bass_utils import run_bass_kernel_spmd

    x2d = np.asarray(x, dtype=np.float32).reshape(TOKENS, IN)
    inv_s, alpha_s = prep_scalars(alpha, act_scale)
    whost, bias_rep = prep_weights(packed_w, bias)

    nc = _build_program(inv_s, alpha_s)

    in_maps = [
        {"x": np.ascontiguousarray(x2d[c * T:(c + 1) * T]),
         "wt": whost, "bias": bias_rep}
        for c in range(N_CORES)
    ]
    res = run_bass_kernel_spmd(nc, in_maps, list(range(N_CORES)),
                               trace=_trace)

    out = np.empty((TOKENS, OUT), dtype=np.float32)
    for c in range(N_CORES):
        out[c * T:(c + 1) * T] = np.asarray(res.results[c]["out"],
                                            dtype=np.float32)
    out = out.reshape(B, S, OUT)
    if _trace:
        return out, res
    return out



# revision 4
# speedup vs baseline: 1.4608x; 1.4608x over previous
"""BitLinear (ternary-weight linear) Trainium2 kernel — on-chip transpose v3.

Math (matching the reference within the 2e-2 gate):
    s   = max(act_scale, 1e-5)
    out = (alpha * s) * (q(x/s) @ sign(W).T) + bias
where q() splits the 32 contraction k-tiles per 128-token block:
  - EX_K exact tiles:   z = round(x/s) int16 (no clip; |z|>127 occurs on
                        ~6e-5 of elements), h = round(z/16), h16 = 16h,
                        l = z - h16. h16 in {-176..176 step 16} and l in
                        [-8,8] are both exact in fp8e4 -> two DoubleRow
                        passes reproduce z exactly.
  - SINGLE_K tiles:     s8 = fp8e4(x/s) directly (no int round, no clip),
                        one DoubleRow pass. rel err measured 1.81e-2 at
                        EX_K=8 against the 2e-2 gate (numpy model).

v1 bottleneck (from TimelineSim): PE idle ~30% waiting on the z DRAM
bounce (quant -> SWDGE store -> transpose-DMA load) whose issue path
saturated the SP sequencer (565ns/DMA config, 900ns DMA sem prop), and
96 MiB/core/rep of HBM traffic. v3 keeps everything on-chip; all PE
transposes are fp8 (walrus rejects int16 Ldweights), so the h16/l/s8
conversion happens token-major BEFORE the transpose:

    x --DMA(ACT ring)--> SBUF --ACT/DVE quant+convert--> h16/l/s8
      token-major fp8 staging --PE transpose (fp8 identity matmul)-->
      PSUM fp8 slabs --DVE/ACT copy--> k-major fp8 tiles --PE DoubleRow
      --> PSUM f32 --DVE drain (x alpha_s + bias, bf16)--> SBUF
      --SWDGE--> out

HBM traffic/core/rep: x 32 MiB + wt 16 MiB + out 16 MiB = 64 MiB.
PE work/rep: 16 blocks x 8 nchunks x 4 q x 20 DR matmuls = 2560 matmuls
(~273us) + 640 transposes (~34us).

Engine plan (in-order sequencers; DMA waits stall the issuing SEQ, so
each ring only carries DMAs whose deps resolve early):
    ACT  : x loads (HWDGE), quant z16/s8, h-round, s8 psum->SBUF copies
    SP   : wt half-tile loads only (HWDGE)
    PE   : transposes + DoubleRow matmuls
    DVE  : h16/l converts, h16/l psum->SBUF copies, psum drains
    Pool : identity setup, osb stores (SWDGE ring)

Cross-rep pipeline: next rep's quant runs during this rep's n-phases
0..4 (hl slot for quarter 0 is the 5th rotating buffer); quarters 1..3
convert at the n=7 tail as their hl slots free (after n7-q matmuls),
with quarter 3 quantized just-in-time to bound SBUF staging at 9
blocks (h16 1KB + l 1KB + s8 3KB per block).
"""

import sys

sys.path.insert(0, "/opt/trn_rl_repo")

import numpy as np
import ml_dtypes

# ---- problem constants (hardcoded per harness contract) ----
B, S, IN, OUT = 4, 4096, 4096, 4096
TOKENS = B * S              # 16384
N_CORES = 8
T = TOKENS // N_CORES       # 2048 tokens per core
KT = IN // 128              # 32 k-tiles (contraction)
EX_K = 8                    # exact k-tiles (h16 + l passes)
SINGLE_K = KT - EX_K        # single-pass k-tiles (direct fp8 quant)
N_CHUNK = 512               # output columns per PSUM tile
NT = OUT // N_CHUNK         # 8 n-chunks
Q = 512                     # token-quarter
NQ = T // Q                 # 4 quarters
MT = Q // 128               # 4 token-blocks per quarter
NBLK = T // 128             # 16 token-blocks
EX_COLS = EX_K * 128        # 1024 leading exact columns
S_COLS = IN - EX_COLS       # 3072 single-pass columns
HKT = KT // 2               # 16 k-tiles per weight half
TG = 2                      # k-tiles per transpose/psum group


def _build_program(inv_s: float, alpha_s: float, reps: int = 1):
    import concourse.mybir as mybir
    import concourse.tile as tile
    from concourse import bacc
    from concourse import masks

    nc = bacc.Bacc("TRN2", target_bir_lowering=False, debug=False,
                   num_devices=N_CORES)

    x_d = nc.dram_tensor("x", [T, IN], mybir.dt.float32, kind="ExternalInput")
    # wt[p, n, k, c] = sign(W)[n*512 + c, k*128 + p]
    wt_d = nc.dram_tensor("wt", [128, NT, KT, N_CHUNK], mybir.dt.float8e4,
                          kind="ExternalInput")
    bias_d = nc.dram_tensor("bias", [128, OUT], mybir.dt.bfloat16,
                            kind="ExternalInput")
    out_d = nc.dram_tensor("out", [NQ, MT, 128, OUT], mybir.dt.bfloat16,
                           kind="ExternalOutput")

    AF = mybir.ActivationFunctionType
    ALU = mybir.AluOpType
    DR = mybir.MatmulPerfMode.DoubleRow

    with tile.TileContext(nc) as tc:
        with (
            tc.tile_pool(name="xp", bufs=2) as xp,
            tc.tile_pool(name="stg", bufs=9) as stg,
            tc.tile_pool(name="z16p", bufs=2) as z16p,
            tc.tile_pool(name="htp", bufs=1) as htp,
            tc.tile_pool(name="hl", bufs=NQ + 1) as hl_pool,
            tc.tile_pool(name="wtp", bufs=3) as wt_pool,
            tc.tile_pool(name="biasp", bufs=1) as bias_pool,
            tc.tile_pool(name="osb", bufs=2) as osb_pool,
            tc.tile_pool(name="idp", bufs=1) as idp,
            tc.tile_pool(name="psmm", bufs=4, space="PSUM") as psum_mm,
            tc.tile_pool(name="psts", bufs=3, space="PSUM") as psum_ts,
        ):
            bias_t = bias_pool.tile([128, OUT], mybir.dt.bfloat16, tag="bias")
            nc.sync.dma_start(bias_t[:], bias_d.ap())
            id8 = idp.tile([128, 128], mybir.dt.float8e4, tag="id8")
            masks.make_identity(nc, id8[:])

            def emit_xq(b):
                """x load (ACT HWDGE) + quantize block b token-major:
                exact cols -> h16/l fp8 pair, single cols -> s8 fp8."""
                r0 = b * 128
                h16s = stg.tile([128, EX_COLS], mybir.dt.float8e4,
                                tag="h16tok")
                ls = stg.tile([128, EX_COLS], mybir.dt.float8e4, tag="ltok")
                s8 = stg.tile([128, S_COLS], mybir.dt.float8e4, tag="s8tok")
                for h in range(2):
                    c0 = h * 2048
                    xt = xp.tile([128, 2048], mybir.dt.float32, tag="x")
                    nc.scalar.dma_start(xt[:],
                                        x_d.ap()[r0:r0 + 128, c0:c0 + 2048])
                    if c0 < EX_COLS:
                        w = EX_COLS - c0
                        z16 = z16p.tile([128, EX_COLS], mybir.dt.int16,
                                        tag="z16")
                        nc.scalar.activation(z16[:, :w], xt[:, :w],
                                             AF.Copy, bias=0.0,
                                             scale=float(inv_s))
                        ht = htp.tile([128, EX_COLS], mybir.dt.int16,
                                      tag="ht")
                        nc.scalar.activation(ht[:, :w], z16[:, :w],
                                             AF.Copy, bias=0.0, scale=0.0625)
                        nc.vector.tensor_scalar(h16s[:, c0:c0 + w],
                                                ht[:, :w], 16.0,
                                                None, ALU.mult)
                        nc.vector.tensor_tensor(ls[:, c0:c0 + w], z16[:, :w],
                                                h16s[:, c0:c0 + w],
                                                ALU.subtract)
                    if c0 + 2048 > EX_COLS:
                        lo = max(EX_COLS - c0, 0)
                        nc.scalar.activation(
                            s8[:, c0 + lo - EX_COLS:c0 + 2048 - EX_COLS],
                            xt[:, lo:], AF.Copy, bias=0.0,
                            scale=float(inv_s))
                return h16s, ls, s8

            def emit_tc(q, stgs, groups, tiles=None):
                """Transpose staged fp8 tiles of quarter q into the k-major
                hl tiles. groups = list of (region, g) with region in
                {0:h16, 1:l, 2:s8} covering k-tile groups of TG."""
                if tiles is None:
                    tiles = (
                        hl_pool.tile([128, EX_K, Q], mybir.dt.float8e4,
                                     tag="h16", name="h16t"),
                        hl_pool.tile([128, EX_K, Q], mybir.dt.float8e4,
                                     tag="l", name="lt"),
                        hl_pool.tile([128, SINGLE_K, Q], mybir.dt.float8e4,
                                     tag="s8", name="s8t"),
                    )
                for region, g in groups:
                    # fp8 transpose psum writes are 16-bit granular (walrus:
                    # "output element step of 2"); the value lands in lane 0.
                    ps = psum_ts.tile([128, TG, Q, 2], mybir.dt.float8e4)
                    for j in range(TG):
                        kt = g * TG + j
                        for mb in range(MT):
                            nc.tensor.transpose(
                                ps[:, j, mb * 128:(mb + 1) * 128, 0],
                                stgs[mb][region][:, kt * 128:(kt + 1) * 128],
                                id8[:])
                    dst = tiles[region][:, g * TG:(g + 1) * TG, :]
                    if region == 2:
                        nc.scalar.activation(dst, ps[:, :, :, 0], AF.Copy,
                                             bias=0.0, scale=1.0)
                    else:
                        nc.vector.tensor_copy(dst, ps[:, :, :, 0])
                return tiles

            ALL_GROUPS = ([(0, g) for g in range(EX_K // TG)] +
                          [(1, g) for g in range(EX_K // TG)] +
                          [(2, g) for g in range(SINGLE_K // TG)])

            def load_wt_half(n, hf):
                wt = wt_pool.tile([128, HKT, N_CHUNK], mybir.dt.float8e4,
                                  tag="wt")
                nc.sync.dma_start(wt[:],
                                  wt_d.ap()[:, n, hf * HKT:(hf + 1) * HKT, :])
                return wt

            def emit_mm(n, q, mm, wta, wtb, hls_q, osb_t):
                h16_t, l_t, s8_t = hls_q
                ms = slice(mm * 128, (mm + 1) * 128)
                psum = psum_mm.tile([128, N_CHUNK], mybir.dt.float32)
                for kk in range(EX_K // 2):
                    ks = slice(2 * kk, 2 * kk + 2)
                    nc.tensor.matmul(psum[:], h16_t[:, ks, ms], wta[:, ks, :],
                                     start=(kk == 0), stop=False,
                                     perf_mode=DR)
                for kk in range(EX_K // 2):
                    ks = slice(2 * kk, 2 * kk + 2)
                    nc.tensor.matmul(psum[:], l_t[:, ks, ms], wta[:, ks, :],
                                     start=False, stop=False, perf_mode=DR)
                for kk in range(SINGLE_K // 2):
                    g0 = EX_K + 2 * kk          # global k-tile
                    ks = slice(2 * kk, 2 * kk + 2)
                    if g0 < HKT:
                        wk, wtile = slice(g0, g0 + 2), wta
                    else:
                        wk, wtile = slice(g0 - HKT, g0 - HKT + 2), wtb
                    nc.tensor.matmul(psum[:], s8_t[:, ks, ms], wtile[:, wk, :],
                                     start=False,
                                     stop=(kk == SINGLE_K // 2 - 1),
                                     perf_mode=DR)
                nc.vector.scalar_tensor_tensor(
                    osb_t[:, mm, :], psum[:], float(alpha_s),
                    bias_t[:, n * N_CHUNK:(n + 1) * N_CHUNK],
                    ALU.mult, ALU.add)

            # ---- prologue: stage + convert all 4 quarters of rep 0 ----
            stgs = [None] * NBLK
            hls = [None] * NQ
            for q in range(NQ):
                for mb in range(MT):
                    stgs[q * MT + mb] = emit_xq(q * MT + mb)
                hls[q] = emit_tc(q, stgs[q * MT:(q + 1) * MT], ALL_GROUPS)

            for rep in range(reps):
                nxt = rep + 1 < reps
                nstgs = [None] * NBLK
                nhls = [None] * NQ
                wta = {0: load_wt_half(0, 0)}
                wtb = {0: load_wt_half(0, 1)}
                wta[1] = load_wt_half(1, 0)
                for n in range(NT):
                    for q in range(NQ):
                        osb_t = osb_pool.tile([128, MT, N_CHUNK],
                                              mybir.dt.bfloat16, tag="osb")
                        for mm in range(MT):
                            emit_mm(n, q, mm, wta[n], wtb[n], hls[q], osb_t)
                        nc.gpsimd.dma_start(
                            out_d.ap()[q, :, :,
                                       n * N_CHUNK:(n + 1) * N_CHUNK]
                            .transpose([1, 0, 2]),
                            osb_t[:])
                        # ---- pipeline hooks (prepare rep+1) ----
                        if n + 1 < NT and q == 0:
                            wtb[n + 1] = load_wt_half(n + 1, 1)
                        if n + 2 < NT and q == 2:
                            wta[n + 2] = load_wt_half(n + 2, 0)
                        if nxt:
                            if n in (0, 2, 4):
                                b = n // 2 * MT + q
                                nstgs[b] = emit_xq(b)
                            if n in (4, 5):
                                gi = 2 * ((n - 4) * NQ + q)  # 0..14 of 20
                                nhls[0] = emit_tc(
                                    0, nstgs[0:MT],
                                    ALL_GROUPS[gi:gi + 2], tiles=nhls[0])
                            if n == 6 and q < 2:
                                nhls[0] = emit_tc(
                                    0, nstgs[0:MT],
                                    ALL_GROUPS[16 + 2 * q:18 + 2 * q],
                                    tiles=nhls[0])
                            if n == 7 and q < 3:
                                qq = q + 1
                                if qq == 3:
                                    for mb in range(MT):
                                        nstgs[3 * MT + mb] = emit_xq(
                                            3 * MT + mb)
                                nhls[qq] = emit_tc(
                                    qq, nstgs[qq * MT:(qq + 1) * MT],
                                    ALL_GROUPS)
                if nxt:
                    stgs, hls = nstgs, nhls

    nc.compile()
    return nc


def prep_scalars(alpha, act_scale):
    s = max(float(np.asarray(act_scale)), 1e-5)
    inv_s = 1.0 / np.float32(s)
    alpha_s = float(np.float32(np.asarray(alpha, dtype=np.float32)) *
                    np.float32(s))
    return float(inv_s), alpha_s


def prep_weights(packed_w, bias):
    """Host-side weight/bias packing (replicated across cores)."""
    w_sign = np.asarray(packed_w, dtype=np.float32) - 1.0     # [OUT, IN]
    # wt[p, n, k, c] = w_sign[n*512 + c, k*128 + p]
    wt = w_sign.reshape(NT, N_CHUNK, KT, 128).transpose(3, 0, 2, 1)
    whost = np.ascontiguousarray(wt.astype(ml_dtypes.float8_e4m3))
    bias_rep = np.ascontiguousarray(
        np.broadcast_to(
            np.asarray(bias, dtype=np.float32).astype(ml_dtypes.bfloat16)
            [None, :], (128, OUT)))
    return whost, bias_rep


def kernel(x, packed_w, alpha, act_scale, bias, _trace=False):
    from concourse.bass_utils import run_bass_kernel_spmd

    x2d = np.asarray(x, dtype=np.float32).reshape(TOKENS, IN)
    inv_s, alpha_s = prep_scalars(alpha, act_scale)
    whost, bias_rep = prep_weights(packed_w, bias)

    nc = _build_program(inv_s, alpha_s)

    in_maps = [
        {"x": np.ascontiguousarray(x2d[c * T:(c + 1) * T]),
         "wt": whost, "bias": bias_rep}
        for c in range(N_CORES)
    ]
    res = run_bass_kernel_spmd(nc, in_maps, list(range(N_CORES)),
                               trace=_trace)

    out = np.empty((TOKENS, OUT), dtype=np.float32)
    for c in range(N_CORES):
        out[c * T:(c + 1) * T] = np.asarray(
            res.results[c]["out"], dtype=np.float32).reshape(T, OUT)
    out = out.reshape(B, S, OUT)
    if _trace:
        return out, res
    return out
